# revision 4
# baseline (speedup 1.0000x reference)
"""Trainium2 Bass kernel for nn_Abstraction (sparse_attention) — v2.

Reference computation (per batch element, N=4096, D=512, A=64):
    c      = l2_normalize(data, axis=-1)
    sim    = tril(c @ c.T)                      # [N, N] never materialized
    pooled = sim.reshape(N, N//A, A).mean(-2)   # [N, A]
    out    = concat([data, pooled @ W_abs], -1) @ W_merge

Identity: pooled[n, a] = (1/A) * c_n . ( sum_{g: g*A+a <= n} c_{g*A+a} )
Per 128-row tile t (blocks q0=2t, q1=2t+1):
    S    = Ct @ Ct^T                    [128,128]  (local sim, PE)
    CPS  = Ct @ PS_t^T + Ct @ base^T    [128,64]   (running group sums + cross-half base)
    pooled_t = (mask-fold of S) + CPS   then per-row scale
    out_t = data_t @ Wm1 + pooled_t @ (W_abs @ Wm2)

v2 design (driven by the CoreSim cost model that is the metric):
 - Weights folded on host: wc16 = W_abs @ Wm2 (f16), wm8 = 32*Wm1 packed
   [ki, ko, o] in fp8e4 (or wm16 = Wm1 in f16 when K_FP8=0).
 - Data loads are f32->f16 SWDGE casts on gpsimd (halves DMA cost; only
   gpsimd can cast).  Stores are f32 on SP.  Xc -> fp8 copy for the merged
   matmul is an SBUF->SBUF DMA cast (no elementwise op).
 - Merged data-part matmuls run fp8 DoubleRow (2 k-chunks per matmul,
   0.5 cyc/row).  Xc holds 16*c so fp8 values are well-scaled; Wm1 carries
   the 32x; the output restore scale is norm/512.
 - Engine balance: Act = squares(own)+sqrt+osb restore; DVE = squares(pre),
   c16/chain/Xc copies/mask/scales; Pool = all cast DMAs + sel_s + fold;
   SP = output stores; PE = matmuls only.
"""

import os
import sys

sys.path.insert(0, "/opt/trn_rl_repo")

import numpy as np

import concourse.bass as bass
import concourse.mybir as mybir
import concourse.tile as tile
from concourse import bacc
from concourse.bass_utils import run_bass_kernel_spmd
from concourse.masks import make_identity

F32 = mybir.dt.float32
F16 = mybir.dt.float16
F8 = mybir.dt.float8e4

FP8 = os.environ.get("K_FP8", "1") == "1"

B, N, D = 4, 4096, 512
A = 64            # abstraction (pool block) size
HALF = N // 2     # rows per core
NT = HALF // 128  # 128-row tiles per core (16)
KC = D // 128     # contraction chunks (4)
EPS = 1e-12

CSCL = 16.0                      # Xc holds CSCL*c
WSCL = 32.0 if FP8 else 1.0      # host premultiplies Wm1 by WSCL
KTOT = CSCL * WSCL               # psum data-part = KTOT * (c @ Wm1)
# pooled path: psum pooled-part = p8 * 256 * raw ; out = (norm/KTOT)*psum
P8 = KTOT / (64.0 * CSCL * CSCL)  # per-row pooled scale multiplies s_n

AL = mybir.AluOpType
AF = mybir.ActivationFunctionType


def _build_nc():
    nc = bacc.Bacc(None)

    xd = nc.dram_tensor("xd", [HALF, D], F32, kind="ExternalInput")
    xp = nc.dram_tensor("xp", [HALF, D], F32, kind="ExternalInput")
    wmd = nc.dram_tensor("wmd", [128, KC, D], F8 if FP8 else F16,
                         kind="ExternalInput")
    wcd = nc.dram_tensor("wcd", [A, D], F16, kind="ExternalInput")
    out = nc.dram_tensor("out", [HALF, D], F32, kind="ExternalOutput")

    with tile.TileContext(nc) as tc:
        with (
            tc.tile_pool(name="persist", bufs=1) as pp,
            tc.tile_pool(name="load", bufs=6) as lp,
            tc.tile_pool(name="work", bufs=4) as wp,
            tc.tile_pool(name="psbig", bufs=3, space="PSUM") as psb,
            tc.tile_pool(name="pstp", bufs=2, space="PSUM") as pst,
            tc.tile_pool(name="pscps", bufs=1, space="PSUM") as psc,
            tc.tile_pool(name="psbase", bufs=1, space="PSUM") as psz,
            nc.allow_low_precision("fp16/fp8 matmul operands by design"),
        ):
            # ---------------- constants ----------------
            ident = pp.tile([128, 128], F16, tag="ident")
            make_identity(nc, ident)

            # mask128: rows 0:64 = [tril64 | 0], rows 64:128 = [ones | tril64]
            mask = pp.tile([128, 128], F16, tag="mask")
            nc.gpsimd.memset(mask, 1.0)
            for hh, base_off in ((0, 0), (1, 64)):
                nc.gpsimd.affine_select(
                    out=mask[hh * 64:(hh + 1) * 64, :],
                    in_=mask[hh * 64:(hh + 1) * 64, :],
                    compare_op=AL.is_ge,
                    fill=0.0,
                    base=base_off,
                    pattern=[[-1, 128]],  # keep when p + base - col >= 0
                    channel_multiplier=1,
                )

            # sel: two stacked 64x64 identities -> [128, 64]
            sel = pp.tile([128, A], F16, tag="sel")
            nc.gpsimd.memset(sel, 0.0)
            for hh in range(2):
                nc.gpsimd.affine_select(
                    out=sel[hh * 64:(hh + 1) * 64, :],
                    in_=sel[hh * 64:(hh + 1) * 64, :],
                    compare_op=AL.not_equal,
                    fill=1.0,
                    base=0,
                    pattern=[[-1, A]],
                    channel_multiplier=1,
                )

            eps_sb = pp.tile([128, 1], F32, tag="eps")
            nc.vector.memset(eps_sb, EPS)

            # ---------------- per-row state ----------------
            sq = pp.tile([128, NT], F32, tag="sq")
            norm = pp.tile([128, NT], F32, tag="norm")
            s16 = pp.tile([128, NT], F32, tag="s16")    # CSCL / norm
            p8 = pp.tile([128, NT], F32, tag="p8")      # P8 / norm
            rK = pp.tile([128, NT], F32, tag="rK")      # norm / KTOT
            sqp = pp.tile([128, NT], F32, tag="sqp")
            normp = pp.tile([128, NT], F32, tag="normp")
            s_pre = pp.tile([128, NT], F32, tag="s_pre")

            # Xc: tile-major transposed scaled rows: Xc[:, t, k, :] = (CSCL*c)^T
            Xc = pp.tile([128, NT, KC, 128], F16, tag="Xc")
            Xc8 = (pp.tile([128, NT, KC, 128], F8, tag="Xc8", name="Xc8")
                   if FP8 else None)
            # PS[:, k, t, :]: group sums (of CSCL*c) before tile t (own half)
            PS = pp.tile([128, KC, NT, A], F16, tag="PS")
            t3 = pp.tile([128, NT, A], F16, tag="t3")     # masked local fold
            ps_nat = pp.tile([128, NT, A], F16, tag="ps_nat")
            pooledT = pp.tile([A, NT, 128], F16, tag="pooledT")
            baseT = pp.tile([128, KC, A], F16, tag="baseT")

            wm_sb = pp.tile([128, KC, D], F8 if FP8 else F16, tag="wm")
            wc_sb = pp.tile([A, D], F16, tag="wc")

            # ---------------- DMA: loads (gpsimd casts) ----------------
            dts, dpts = {}, {}
            for t in range(NT):
                dt_ = lp.tile([128, D], F16, tag="dt", name=f"dt{t}")
                nc.gpsimd.dma_start(out=dt_, in_=xd[t * 128:(t + 1) * 128, :])
                dts[t] = dt_
                dpt = lp.tile([128, D], F16, tag="dpt", name=f"dpt{t}")
                nc.gpsimd.dma_start(out=dpt, in_=xp[t * 128:(t + 1) * 128, :])
                dpts[t] = dpt
            nc.gpsimd.dma_start(out=wm_sb, in_=wmd[:, :, :])
            nc.gpsimd.dma_start(out=wc_sb, in_=wcd[:, :])

            cps = psc.tile([128, NT, A], F32, tag="cps", name="cpsall")
            base_ps = psz.tile([A, D], F32, tag="base", name="base_ps")

            nc.vector.memset(PS[:, :, 0, :], 0.0)

            # ---------------- per-tile pipelines ----------------
            def own_tile(t):
                dt_ = dts[t]
                scr = wp.tile([128, D], F16, tag="scr")
                nc.scalar.activation(
                    out=scr, in_=dt_, func=AF.Square,
                    accum_out=sq[:, t:t + 1],
                )
                nc.scalar.activation(
                    out=norm[:, t:t + 1], in_=sq[:, t:t + 1],
                    func=AF.Sqrt, bias=eps_sb,
                )
                nc.vector.reciprocal(out=s16[:, t:t + 1], in_=norm[:, t:t + 1])
                # s16 currently 1/norm; scale to CSCL/norm and derive others
                nc.vector.tensor_scalar_mul(p8[:, t:t + 1], s16[:, t:t + 1],
                                            P8)
                nc.vector.tensor_scalar_mul(rK[:, t:t + 1], norm[:, t:t + 1],
                                            1.0 / KTOT)
                nc.vector.tensor_scalar_mul(s16[:, t:t + 1], s16[:, t:t + 1],
                                            CSCL)
                c16 = wp.tile([128, D], F16, tag="c16")
                nc.vector.tensor_scalar_mul(c16, dt_, s16[:, t:t + 1])

                tp = pst.tile([128, KC, 128], F16, tag="tp", name=f"tp{t}")
                for k in range(KC):
                    nc.tensor.transpose(
                        tp[:, k, :], c16[:, k * 128:(k + 1) * 128], ident
                    )
                nc.vector.tensor_copy(out=Xc[:, t, :, :], in_=tp)
                if FP8:
                    nc.gpsimd.dma_start(out=Xc8[:, t, :, :], in_=Xc[:, t, :, :])

                # chain: PS_{t+1} = PS_t + block(2t) + block(2t+1)
                if t + 1 < NT:
                    tmp = wp.tile([128, KC, A], F16, tag="chtmp")
                    nc.vector.tensor_tensor(
                        tmp, Xc[:, t, :, 0:A], Xc[:, t, :, A:2 * A], AL.add
                    )
                    nc.vector.tensor_tensor(
                        PS[:, :, t + 1, :], PS[:, :, t, :], tmp, AL.add
                    )

                # local sim S and CPS (running sums part)
                S = psb.tile([128, 128], F32, tag="big", name=f"S{t}")
                for k in range(KC):
                    nc.tensor.matmul(
                        S, Xc[:, t, k, :], Xc[:, t, k, :],
                        start=(k == 0), stop=(k == KC - 1),
                    )
                for k in range(KC):
                    nc.tensor.matmul(
                        cps[:, t, :], Xc[:, t, k, :], PS[:, k, t, :],
                        start=(k == 0), stop=False, skip_group_check=True,
                    )
                # mask + fold (frees S quickly)
                M = wp.tile([128, 128], F16, tag="M")
                nc.vector.tensor_tensor(M, S, mask, AL.mult)
                nc.gpsimd.scalar_tensor_tensor(
                    out=t3[:, t, :], in0=M[:, 0:A], scalar=1.0,
                    in1=M[:, A:2 * A], op0=AL.mult, op1=AL.add,
                )

            def prefix_tile(t):
                dpt = dpts[t]
                scr2 = wp.tile([128, D], F16, tag="scr2")
                nc.vector.scalar_tensor_tensor(
                    out=scr2, in0=dpt, scalar=1.0, in1=dpt,
                    op0=AL.mult, op1=AL.mult,
                    accum_out=sqp[:, t:t + 1],
                )
                nc.scalar.activation(
                    out=normp[:, t:t + 1], in_=sqp[:, t:t + 1],
                    func=AF.Sqrt, bias=eps_sb,
                )
                nc.vector.reciprocal(out=s_pre[:, t:t + 1],
                                     in_=normp[:, t:t + 1])
                sel_s = wp.tile([128, A], F16, tag="sel_s")
                nc.gpsimd.tensor_scalar_mul(sel_s, sel, s_pre[:, t:t + 1])
                nc.tensor.matmul(
                    base_ps, sel_s, dpt, start=(t == 0), stop=(t == NT - 1),
                )

            for t in range(NT):
                own_tile(t)
                prefix_tile(t)

            # ---------------- base -> baseT (scaled by CSCL) ----------------
            base_sb = pp.tile([A, D], F16, tag="base_sb")
            nc.scalar.activation(out=base_sb, in_=base_ps, func=AF.Copy,
                                 scale=CSCL)
            btp = pst.tile([128, KC, A], F16, tag="tp", name="btp")
            for k in range(KC):
                nc.tensor.transpose(
                    btp[:, k, :], base_sb[:, k * 128:(k + 1) * 128],
                    ident[0:A, 0:A],
                )
            nc.vector.tensor_copy(out=baseT, in_=btp)

            # ---------------- phase B ----------------
            for t in range(NT):
                # cross-half base contribution closes the cps accumulation
                for k in range(KC):
                    nc.tensor.matmul(
                        cps[:, t, :], Xc[:, t, k, :], baseT[:, k, :],
                        start=False, stop=(k == KC - 1),
                        skip_group_check=True,
                    )
                t4 = wp.tile([128, A], F16, tag="t4")
                nc.vector.scalar_tensor_tensor(
                    out=t4, in0=cps[:, t, :], scalar=1.0, in1=t3[:, t, :],
                    op0=AL.mult, op1=AL.add,
                )
                nc.vector.tensor_scalar_mul(
                    ps_nat[:, t, :], t4, p8[:, t:t + 1]
                )
                ptp = pst.tile([A, 128], F16, tag="tp", name=f"ptp{t}")
                nc.tensor.transpose(ptp, ps_nat[:, t, :], ident)
                nc.vector.tensor_copy(out=pooledT[:, t, :], in_=ptp)

                mg = psb.tile([128, D], F32, tag="big", name=f"mg{t}")
                if FP8:
                    for j in range(2):
                        nc.tensor.matmul(
                            mg, Xc8[:, t, 2 * j:2 * j + 2, :],
                            wm_sb[:, 2 * j:2 * j + 2, :],
                            start=(j == 0), stop=False,
                            perf_mode=mybir.MatmulPerfMode.DoubleRow,
                        )
                else:
                    for k in range(KC):
                        nc.tensor.matmul(
                            mg, Xc[:, t, k, :], wm_sb[:, k, :],
                            start=(k == 0), stop=False,
                        )
                nc.tensor.matmul(
                    mg, pooledT[:, t, :], wc_sb, start=False, stop=True,
                )
                osb = wp.tile([128, D], F32, tag="osb")
                nc.scalar.activation(
                    out=osb, in_=mg, func=AF.Copy, scale=rK[:, t:t + 1],
                )
                nc.sync.dma_start(out=out[t * 128:(t + 1) * 128, :], in_=osb)

    nc.finalize()
    return nc


_NC_CACHE = None


def _get_nc():
    global _NC_CACHE
    if _NC_CACHE is None:
        _NC_CACHE = _build_nc()
    return _NC_CACHE


def _host_weights(W_abs, W_merge):
    """Fold weights on host: wm = WSCL*Wm1 packed [ki, ko, o]; wc = Wabs@Wm2."""
    Wm1 = W_merge[0:D, :]
    Wm2 = W_merge[D:2 * D, :]
    wm = (Wm1 * WSCL).reshape(KC, 128, D).transpose(1, 0, 2)
    if FP8:
        import ml_dtypes
        wm = np.ascontiguousarray(wm).astype(ml_dtypes.float8_e4m3)
    else:
        wm = np.ascontiguousarray(wm).astype(np.float16)
    wc = np.ascontiguousarray((W_abs @ Wm2).astype(np.float16))
    return wm, wc


_RUNNER = None


def _get_runner():
    """Build (once) a cached jitted SPMD executor for the 8-core kernel."""
    global _RUNNER
    if _RUNNER is not None:
        return _RUNNER

    import jax
    from jax.sharding import Mesh, PartitionSpec
    from jax.experimental.shard_map import shard_map

    import concourse.mybir as mybir
    from concourse import bass2jax

    bass2jax.install_neuronx_cc_hook()
    nc = _get_nc()
    n_cores = 8

    partition_name = (
        nc.partition_id_tensor.name if nc.partition_id_tensor else None
    )
    in_names, out_names, out_shapes, out_dtypes, zero_outs = [], [], [], [], []
    for alloc in nc.m.functions[0].allocations:
        if not isinstance(alloc, mybir.MemoryLocationSet):
            continue
        name = alloc.memorylocations[0].name
        if alloc.kind == "ExternalInput":
            if name != partition_name:
                in_names.append(name)
        elif alloc.kind == "ExternalOutput":
            shape = tuple(alloc.tensor_shape)
            dtype = mybir.dt.np(alloc.dtype)
            out_names.append(name)
            out_shapes.append(shape)
            out_dtypes.append(dtype)
            zero_outs.append(np.zeros(shape, dtype))
    n_params = len(in_names)
    out_avals = [
        jax.core.ShapedArray(s, d) for s, d in zip(out_shapes, out_dtypes)
    ]
    all_in_names = list(in_names) + list(out_names)
    if partition_name is not None:
        all_in_names.append(partition_name)
    donate = tuple(range(n_params, n_params + len(out_names)))

    def _body(*args):
        operands = list(args)
        if partition_name is not None:
            operands.append(bass2jax.partition_id_tensor())
        outs = bass2jax._bass_exec_p.bind(
            *operands,
            out_avals=tuple(out_avals),
            in_names=tuple(all_in_names),
            out_names=tuple(out_names),
            lowering_input_output_aliases=(),
            sim_require_finite=True,
            sim_require_nnan=True,
            nc=nc,
        )
        return tuple(outs)

    devices = jax.devices()[:n_cores]
    mesh = Mesh(np.asarray(devices), ("core",))
    in_specs = (PartitionSpec("core"),) * (n_params + len(out_names))
    out_specs = (PartitionSpec("core"),) * len(out_names)
    sharded = jax.jit(
        shard_map(
            _body, mesh=mesh, in_specs=in_specs, out_specs=out_specs,
            check_rep=False,
        ),
        donate_argnums=donate,
        keep_unused=True,
    )
    _RUNNER = (sharded, in_names, out_names, out_shapes, zero_outs, n_cores)
    return _RUNNER


def _run_fast(in_maps):
    sharded, in_names, out_names, out_shapes, zero_outs, n_cores = _get_runner()
    concat_in = [
        np.concatenate([in_maps[c][nm] for c in range(n_cores)], axis=0)
        for nm in in_names
    ]
    big_zeros = [
        np.zeros((n_cores * z.shape[0],) + z.shape[1:], z.dtype)
        for z in zero_outs
    ]
    out_arrs = sharded(*concat_in, *big_zeros)
    return [
        {
            nm: np.asarray(out_arrs[i]).reshape(
                (n_cores,) + out_shapes[i])[c]
            for i, nm in enumerate(out_names)
        }
        for c in range(n_cores)
    ]


def make_in_maps(data, W_abs, W_merge):
    wm, wc = _host_weights(W_abs, W_merge)
    zeros_half = np.zeros((HALF, D), np.float32)
    in_maps = []
    for core in range(8):
        b, h = divmod(core, 2)
        in_maps.append({
            "xd": np.ascontiguousarray(data[b, h * HALF:(h + 1) * HALF]),
            "xp": np.ascontiguousarray(data[b, 0:HALF]) if h == 1 else zeros_half,
            "wmd": wm,
            "wcd": wc,
        })
    return in_maps


def kernel(data, W_abs, W_merge, _trace=False):
    data = np.ascontiguousarray(np.asarray(data, dtype=np.float32))
    W_abs = np.ascontiguousarray(np.asarray(W_abs, dtype=np.float32))
    W_merge = np.ascontiguousarray(np.asarray(W_merge, dtype=np.float32))
    assert data.shape == (B, N, D)

    in_maps = make_in_maps(data, W_abs, W_merge)

    if _trace:
        nc = _get_nc()
        res = run_bass_kernel_spmd(
            nc, in_maps, core_ids=list(range(8)), trace=True,
            stitch_traces=True,
        )
        results = res.results
    else:
        res = None
        results = _run_fast(in_maps)

    out = np.empty((B, N, D), np.float32)
    for core in range(8):
        b, h = divmod(core, 2)
        out[b, h * HALF:(h + 1) * HALF] = results[core]["out"]
    if _trace:
        return out, res
    return out
